# revision 12
# baseline (speedup 1.0000x reference)
"""Trainium2 Bass kernel for BaseXRayVolumeRenderer.

Full-input contract: kernel(**inputs) takes the unsharded inputs and returns
the full [1,1,256,256] output. Internally shards the 256x256 pixel grid
across 8 NeuronCores (4 row-blocks x 2 col-blocks).

Math: with R = I the trilinear sampling is separable per depth sample p:
    S_p = A_p^T @ (wz0*vol[z0] + wz1*vol[z1]) @ B_p
The z-blend is host-precomputed per depth sample (z0 is strictly increasing,
so each (z0, z0+1) slice pair belongs to exactly one p); the blended slice
and the A_p interp matrix are packed side by side in one "va" tensor, so
stage 1 is a single K=42 matmul per sample and stage 2 a single K=65 matmul.

Emission-absorption weights: per depth sample p and per core tile, the
absorption field absorption_p(i,j) is approximated rank-1 as r_p(i)*c_p(j)
(validated: max gray error 1.3e-4 against a 0.07 gray range).  r_p folds
into the A-side scale (with sy/192) and c_p into the B-side scale (with sx),
so ALL 65 stage-2 matmuls accumulate into a single PSUM tile:
    gray = opac/4 + pacc.
This removes the vb tensor, the per-8-block SVD, and all per-block vector
fold work of the previous design.

Layout: frustum slicing - each core only loads the vol rows/cols its rays
touch (ny=42 of 128 y-rows, nx=65 of 128 x-cols): ~1.8MB HBM per core,
streamed in 6 geometric waves per tensor alternating across the two HWDGE
rings (sync/scalar); opac rides the SWDGE ring.  The PE clock is pinned at
1.2 GHz on this platform (verified: 4us of back-to-back matmuls never
un-throttles HAM), so no warmup matmuls are issued.

The global standardize+normalize reduces to out = (gray-gmin)/(gmax-gmin)
(the reference's 1e-8-epsilon terms contribute O(1e-9)).  In-kernel
AllReduce costs ~20us+ (mesh collective floor) and remote_dma crashes on
this platform, so per-core per-row min/max go to the host, which combines
8x64 values and launches a tiny second NEFF applying the affine to the f16
gray handoff.
"""

import numpy as np

import concourse.bass as bass
import concourse.bacc as bacc
import concourse.mybir as mybir
import concourse.tile as tile
from concourse.bass_utils import run_bass_kernel_spmd

F32 = mybir.dt.float32
F16 = mybir.dt.float16
ALU = mybir.AluOpType

IMG_H = 256
IMG_W = 256
N_PTS = 192
MIN_DEPTH, MAX_DEPTH, FOCAL = 3.0, 9.0, 4.0
EPS, EA_EPS = 1e-8, 1e-10
GRID = 128
N_CORES = 8
IB, JB = 64, 128            # per-core pixel block: 64 rows x 128 cols
NY, NX = 42, 65             # per-core vol window (y rows, x cols)
GRP = 13                    # depth samples per stage-1 PSUM tile
# DMA wave schedule: (kind, p0, p1) in issue order per ring.  The two HWDGE
# rings (sync/scalar) carry the head and tail; the SWDGE ring (gpsimd)
# carries opac + the middle waves, adding a third ~124GB/s channel.
SCHED = {
    "sync":   (("va", 0, 2), ("va", 2, 8), ("bt", 18, 34),
               ("bt", 34, 48), ("bt", 48, 65)),
    "scalar": (("bt", 0, 2), ("bt", 2, 8), ("va", 18, 34),
               ("va", 34, 48), ("va", 48, 65)),
    "gpsimd": (("op", 0, 0), ("va", 8, 18), ("bt", 8, 18)),
}


def _interp_matrix(f):
    """f: [P, M] voxel coords -> [P, GRID, M] relu(1-|f-k|) interp weights."""
    k = np.arange(GRID, dtype=np.float64)[None, :, None]
    return np.maximum(0.0, 1.0 - np.abs(f[:, None, :] - k))


def _host_geometry(R, T):
    R = np.asarray(R, np.float64)
    T = np.asarray(T, np.float64)[0]
    assert np.allclose(R[0], np.eye(3), atol=1e-5), "kernel assumes R == I"
    ys = np.linspace(1.0, -1.0, IMG_H)
    xs = np.linspace(1.0, -1.0, IMG_W)
    d = np.linspace(MIN_DEPTH, MAX_DEPTH, N_PTS)
    fx = ((xs[None, :] * d[:, None] / FOCAL - T[0]) + 1.0) * 0.5 * (GRID - 1)
    fy = ((ys[None, :] * d[:, None] / FOCAL - T[1]) + 1.0) * 0.5 * (GRID - 1)
    fz = ((d - T[2]) + 1.0) * 0.5 * (GRID - 1)
    zf = np.floor(fz)
    wz = fz - zf
    z0 = np.clip(zf, 0, GRID - 1).astype(np.int64)
    wz0 = (1.0 - wz) * ((zf >= 0) & (zf <= GRID - 1))
    wz1 = wz * ((zf + 1 >= 0) & (zf + 1 <= GRID - 1))
    sz = wz0 + wz1
    active = np.nonzero(sz > 0)[0]
    assert len(active) and active[0] == 0 and np.all(np.diff(active) == 1), \
        "active depth samples must be a prefix for the prefix-cumprod fold"
    P = len(active)
    assert np.all(np.diff(z0[:P]) >= 1), "blend assumes strictly increasing z0"
    Ay = _interp_matrix(fy)[:P]          # [P, 128y, 256i]
    Bx = _interp_matrix(fx)[:P]          # [P, 128x, 256j]
    sy = Ay.sum(axis=1)                  # [P, 256]
    sx = Bx.sum(axis=1)
    dens = (sy[:, :, None] * sx[:, None, :]) * (sz[:P, None, None] / N_PTS)
    t = (1.0 + EA_EPS) - dens
    cp = np.cumprod(t, axis=0)
    absorption = np.concatenate([np.ones_like(cp[:1]), cp[:-1]], axis=0)
    opac4 = 0.25 * (1.0 - np.prod(1.0 - dens, axis=0))  # [H, W]
    # G_p = 0.75*sz_p*absorption_p ~= r_p(i)*c_p(j) rank-1 PER (p, core tile)
    G = 0.75 * sz[:P, None, None] * absorption          # [P, H, W]
    rr = np.zeros((P, 4, 2, IB))
    cc = np.zeros((P, 4, 2, JB))
    for r in range(4):
        for cb in range(2):
            Gt = G[:, r * IB:(r + 1) * IB, cb * JB:(cb + 1) * JB]
            u = np.ones((P, IB))
            for _ in range(4):
                u /= np.linalg.norm(u, axis=1, keepdims=True)
                v = np.einsum("pij,pi->pj", Gt, u)
                u = np.einsum("pij,pj->pi", Gt, v)
            u /= np.abs(u).max(axis=1, keepdims=True)
            v = (np.einsum("pij,pi->pj", Gt, u)
                 / (u * u).sum(axis=1, keepdims=True))
            rr[:, r, cb] = u
            cc[:, r, cb] = v
    a_scale = sy / N_PTS                                  # [P, 256] (i)
    b_scale = sx                                          # [P, 256] (j)
    # per-block vol windows (rows: 4 blocks of 64, cols: 2 blocks of 128)
    row_wins, col_wins = [], []
    for r in range(4):
        nz = np.nonzero(Ay[:, :, r * IB:(r + 1) * IB].sum(axis=(0, 2)) > 0)[0]
        lo = min(int(nz[0]), GRID - NY)
        assert int(nz[-1]) < lo + NY
        row_wins.append(lo)
    for c in range(2):
        nz = np.nonzero(Bx[:, :, c * JB:(c + 1) * JB].sum(axis=(0, 2)) > 0)[0]
        lo = min(int(nz[0]), GRID - NX)
        assert int(nz[-1]) < lo + NX
        col_wins.append(lo)
    return dict(P=P, Ay=Ay, Bx=Bx, z0=[int(z) for z in z0[:P]],
                wz0=wz0[:P], wz1=wz1[:P], a_scale=a_scale, b_scale=b_scale,
                rr=rr, cc=cc, opac4=opac4,
                row_wins=row_wins, col_wins=col_wins)


def _build_nc(P):
    """Build the SPMD Bass program (geometry-independent: host pre-blends)."""
    nc = bacc.Bacc(num_devices=N_CORES)
    W = NX + IB                           # 129 cols per p in va
    va_d = nc.declare_dram_parameter("va", [NY, P * W], F16, isOutput=False)
    bt_d = nc.declare_dram_parameter("bt", [NX, P * JB], F16, isOutput=False)
    op_d = nc.declare_dram_parameter("op4", [IB, JB], F32, isOutput=False)
    out_d = nc.declare_dram_parameter("out", [IB, JB + 4], F16, isOutput=True)

    with tile.TileContext(nc) as tc:
        with tc.tile_pool(name="big", bufs=1) as big:
            # partition placement: partitions 0..63 map to the 8 even SDMA
            # engines, 64..127 to the 8 odd ones. bt (stage-2 rhs, K=65,
            # forced to base 0) rides the evens; va and all pixel-row
            # tensors sit at base 64 so their DMAs ride the odds.
            va_sb = big.tile([64 + NY, P * W], F16)
            bt_sb = big.tile([NX, P * JB], F16)
            op_sb = big.tile([64 + IB, JB], F32)
            gray16_t = big.tile([64 + IB, JB + 4], F16)

            # --- streamed loads per SCHED (3 parallel DMA rings)
            rings = {"sync": nc.sync, "scalar": nc.scalar,
                     "gpsimd": nc.gpsimd}
            for rname, waves in SCHED.items():
                eng = rings[rname]
                for kind, p0, p1 in waves:
                    if kind == "op":
                        eng.dma_start(op_sb[64:64 + IB, :], op_d[:])
                    elif kind == "va":
                        eng.dma_start(va_sb[64:64 + NY, p0 * W:p1 * W],
                                      va_d[:, p0 * W:p1 * W])
                    else:
                        eng.dma_start(bt_sb[:, p0 * JB:p1 * JB],
                                      bt_d[:, p0 * JB:p1 * JB])

            # --- main loop: groups of GRP depth samples share one stage-1
            # PSUM tile; every stage-2 matmul accumulates into ONE pacc tile.
            with tc.tile_pool(name="psY", bufs=3, space="PSUM") as psY, \
                 tc.tile_pool(name="psAcc", bufs=1, space="PSUM") as psAcc, \
                 tc.tile_pool(name="work", bufs=4) as work:
                pacct = psAcc.tile([64 + IB, JB], F32, name="pacc")
                pacc = pacct[64:64 + IB, :]

                groups = [list(range(s, min(s + GRP, P)))
                          for s in range(0, P, GRP)]
                # Software-pipelined emission: stage-1 of group g and its
                # cast are emitted BEFORE stage-2 of group g-1, so the PE
                # (strict FIFO) runs s1(g) while the vector cast of g-1 is
                # still in flight, and never idles waiting for a cast.
                ysbs = [None] * len(groups)

                def emit_s1(gi):
                    grp = groups[gi]
                    py = psY.tile([NX, len(grp) * IB], F32, tag="py",
                                  name=f"py{gi}")
                    for k, kk in enumerate(grp):
                        nc.tensor.matmul(
                            py[:, k * IB:(k + 1) * IB],
                            va_sb[64:64 + NY, kk * W:kk * W + NX],
                            va_sb[64:64 + NY, kk * W + NX:(kk + 1) * W],
                            start=True, stop=True,
                            tile_position=(64, 0))
                    ysb = work.tile([NX, len(grp) * IB], F16, tag="ysb",
                                    name=f"ysb{gi}")
                    nc.vector.tensor_copy(ysb[:], py[:])
                    ysbs[gi] = ysb

                def emit_s2(gi):
                    for k, kk in enumerate(groups[gi]):
                        nc.tensor.matmul(pacc[:],
                                         ysbs[gi][:, k * IB:(k + 1) * IB],
                                         bt_sb[:, kk * JB:(kk + 1) * JB],
                                         start=(kk == 0), stop=(kk == P - 1),
                                         tile_position=(0, 64))

                emit_s1(0)
                for gi in range(1, len(groups)):
                    emit_s1(gi)
                    emit_s2(gi - 1)
                emit_s2(len(groups) - 1)

            # --- gray = opac/4 + pacc; stats ride in 4 extra f16 columns of
            # the output (bit-cast f32 pairs); host reduces 8x64 values.
            with tc.tile_pool(name="st", bufs=1) as st:
                rowmm = st.tile([64 + IB, 2], F32)
                nc.vector.tensor_add(gray16_t[64:64 + IB, 0:JB], pacc[:],
                                     op_sb[64:64 + IB, :])
                nc.sync.dma_start(out_d[:, 0:JB], gray16_t[64:64 + IB, 0:JB])
                nc.vector.tensor_reduce(rowmm[64:64 + IB, 0:1],
                                        gray16_t[64:64 + IB, 0:JB],
                                        axis=mybir.AxisListType.X, op=ALU.min)
                nc.vector.tensor_reduce(rowmm[64:64 + IB, 1:2],
                                        gray16_t[64:64 + IB, 0:JB],
                                        axis=mybir.AxisListType.X, op=ALU.max)
                nc.vector.tensor_copy(gray16_t[64:64 + IB, JB:JB + 4],
                                      rowmm[64:64 + IB, :].bitcast(F16))
                nc.scalar.dma_start(out_d[:, JB:JB + 4],
                                    gray16_t[64:64 + IB, JB:JB + 4])
    nc.finalize()
    return nc


def _build_affine():
    """Tiny second NEFF: out = a*gray + b per pixel (a,b host-reduced)."""
    nc = bacc.Bacc(num_devices=N_CORES)
    gray_d = nc.declare_dram_parameter("gray", [IB, JB + 4], F16,
                                       isOutput=False)
    out_d = nc.declare_dram_parameter("out", [IB, JB], F16, isOutput=True)
    with tile.TileContext(nc) as tc:
        with tc.tile_pool(name="aff", bufs=1) as pool:
            gsb = pool.tile([64 + IB, JB + 4], F16)
            osb = pool.tile([64 + IB, JB], F16)
            nc.sync.dma_start(gsb[64:64 + IB, :], gray_d[:])
            ab = gsb[64:64 + IB, JB:JB + 4].bitcast(F32)
            nc.vector.tensor_scalar(osb[64:64 + IB, :],
                                    gsb[64:64 + IB, 0:JB],
                                    ab[:, 0:1], ab[:, 1:2],
                                    ALU.mult, ALU.add)
            nc.sync.dma_start(out_d[:], osb[64:64 + IB, :])
    nc.finalize()
    return nc


_CACHE = {}


def _get_program(geom):
    key = geom["P"]
    if key not in _CACHE:
        _CACHE[key] = _build_nc(geom["P"])
    return _CACHE[key]


def _in_maps(image3d, geom):
    vol = np.asarray(image3d, np.float64)[0, 0]           # [z, y, x]
    volp = np.concatenate([vol, np.zeros((1, GRID, GRID))], axis=0)
    P = geom["P"]
    z0 = np.asarray(geom["z0"])
    W = NX + IB
    maps = []
    for c in range(N_CORES):
        r, cb = c // 2, c % 2
        i0, j0 = r * IB, cb * JB
        ylo = geom["row_wins"][r]
        xlo = geom["col_wins"][cb]
        # blended vol slices [P, NY, NX] + interp [P, NY, IB] side by side
        blend = (geom["wz0"][:, None, None]
                 * volp[z0, ylo:ylo + NY, xlo:xlo + NX]
                 + geom["wz1"][:, None, None]
                 * volp[z0 + 1, ylo:ylo + NY, xlo:xlo + NX])
        at1 = (geom["Ay"][:, ylo:ylo + NY, i0:i0 + IB]
               * (geom["a_scale"][:, None, i0:i0 + IB]
                  * geom["rr"][:, r, cb][:, None, :]))
        va = np.concatenate([blend, at1], axis=2)         # [P, NY, W]
        va_c = np.ascontiguousarray(
            va.transpose(1, 0, 2).reshape(NY, P * W)).astype(np.float16)
        bx = geom["Bx"][:, xlo:xlo + NX, j0:j0 + JB]
        bt = bx * (geom["b_scale"][:, None, j0:j0 + JB]
                   * geom["cc"][:, r, cb][:, None, :])
        bt_c = np.ascontiguousarray(
            bt.transpose(1, 0, 2).reshape(NX, P * JB)).astype(np.float16)
        op_c = np.ascontiguousarray(
            geom["opac4"][i0:i0 + IB, j0:j0 + JB]).astype(np.float32)
        maps.append({"va": va_c, "bt": bt_c, "op4": op_c})
    return maps


def run_kernel(image3d, R, T, trace=False):
    geom = _host_geometry(R, T)
    nc = _get_program(geom)
    maps = _in_maps(image3d, geom)
    res = run_bass_kernel_spmd(nc, maps, list(range(N_CORES)), trace=trace)
    stats = np.stack(
        [np.ascontiguousarray(res.results[c]["out"][:, JB:JB + 4])
             .view(np.float32) for c in range(N_CORES)])
    gmin = float(stats[:, :, 0].min())
    gmax = float(stats[:, :, 1].max())
    a = 1.0 / (gmax - gmin)
    b = -gmin * a
    ab64 = np.tile(np.array([[a, b]], np.float32).view(np.float16), (IB, 1))
    if "affine" not in _CACHE:
        _CACHE["affine"] = _build_affine()
    nc2 = _CACHE["affine"]
    maps2 = []
    for c in range(N_CORES):
        g = np.array(res.results[c]["out"], np.float16)
        g[:, JB:JB + 4] = ab64
        maps2.append({"gray": g})
    res2 = run_bass_kernel_spmd(nc2, maps2, list(range(N_CORES)), trace=trace)
    out = np.zeros((1, 1, IMG_H, IMG_W), np.float32)
    for c in range(N_CORES):
        i0 = (c // 2) * IB
        j0 = (c % 2) * JB
        out[0, 0, i0:i0 + IB, j0:j0 + JB] = res2.results[c]["out"]
    return out, (res, res2)


def kernel(image3d, R, T):
    out, _ = run_kernel(image3d, R, T, trace=False)
    return out
